# revision 45
# baseline (speedup 1.0000x reference)
"""DSNAS MoE-routing forward kernel for 8 Trainium2 NeuronCores.

Computation (see reference): for each of 28 column pairs (i,j), with hard
top-1 routing l = argmax(log_alpha[k]):
    p = M[i] + S01[i]*noise[k,0],  q = M[j] + S01[j]*noise[k,1]
    out += branch_l(p, q) @ W_l.T
where M = emb_mean gathered by features, S01 = softplus(emb_std)*0.01 gathered.

Strategy: data-parallel over batch B=8192 -> 1024 rows per core, tables
replicated.  On device everything lives in [D=128 partitions, B free] layout;
noise is transposed on host during input marshaling.  Embedding gathers happen
on device as one-hot matmuls (one-hot built on host from the int features).
The per-pair branch is specialized at trace time from the actual log_alpha
values passed to kernel(), so the compiled program is always correct for the
inputs it runs on.

Precision: noise ships as bf16 and the noise term t = S01*noise is computed in
bf16 (2x DVE mode).  The noise term is scaled by 0.01, so bf16 rounding there
perturbs the output by only ~1e-5 relative.  fp32 matmuls are 2-pass on TRN2,
so all gather matmuls run in bf16: the one-hot is exact in bf16, S01 tables
are bf16 (error suppressed by 0.01), and emb_mean is gathered as hi+lo bf16
tables accumulated in fp32 PSUM (residual ~1.6e-5 relative).  Only the final
combo projections (mul/max/min pairs) are fp32 matmuls.

Branch algebra: for l=0 (p+q) and l=4 (concat), out = p@Wp + q@Wq distributes
into t0@Wp + t1@Wq (bf16 matmuls) plus a per-column mean-path term
onehot_c @ CM_c, where CM_c sums Mtab_c @ Wpart over every decomposed pair
membership of column c (hi+lo bf16).  Those pairs never materialize p/q.
"""

import os
import sys

import numpy as np
import ml_dtypes

for _p in ("/opt/trn_rl_repo",):
    if _p not in sys.path and os.path.isdir(_p):
        sys.path.insert(0, _p)

import concourse.bacc as bacc
import concourse.bass as bass
import concourse.mybir as mybir
import concourse.tile as tile
from concourse.bass_utils import run_bass_kernel_spmd

COLS = 8
D = 128
B = 8192
NUM_EMB = 12
PAIRS = [(i, j) for i in range(COLS) for j in range(COLS) if i < j]
NPAIR = len(PAIRS)  # 28
NCORES = 8
BS = B // NCORES  # 1024 per core
CH = 512  # matmul free-dim chunk (one PSUM bank of fp32)
NCH = BS // CH

FP32 = mybir.dt.float32
BF16 = mybir.dt.bfloat16
BF = ml_dtypes.bfloat16

_ALU = [
    mybir.AluOpType.add,
    mybir.AluOpType.mult,
    mybir.AluOpType.max,
    mybir.AluOpType.min,
]

# debug switches
DECOMP = os.environ.get("KV_DECOMP", "1") == "1"  # matmul-decompose l in {0,4}
GPS_COMBO = os.environ.get("KV_GPS", "0") == "1"  # combo ops on GpSimd (walrus rejects)
WARMUP = int(os.environ.get("KV_WARMUP", "0"))  # junk matmuls to warm HAM

# cbf (bf16, [NUM_EMB, CBW]) column layout:
#   [MHI0 + c*D ...)   emb_mean col c, bf16 high part
#   [MLO0 + c*D ...)   emb_mean col c, bf16 residual
#   [S0  + c*D ...)    s01 col c
#   [OH0 + c*BS ...)   onehot col c
MHI0 = 0
MLO0 = COLS * D
S0 = 2 * COLS * D
OH0 = 3 * COLS * D
CBW = OH0 + COLS * BS

# oh96 (bf16, [COLS*NUM_EMB, BS + 4]): rows c*12+e = onehot col c; the last
# 4 columns hold the stacked CM tables [hi(2) | lo(2)] so the whole
# decomposed-pair mean path is ONE matmul per output chunk per hi/lo part.
OHW = BS + 4


def _build_program(pos):
    """Build the per-core Bass/Tile program, specialized on routing `pos`."""
    nc = bacc.Bacc("TRN2", target_bir_lowering=False, debug=False)

    # [NPAIR, D, 2, BS]: per-pair slice [D, 2, BS] DMA-flattens into an SBUF
    # tile [D, 2*BS] with matching element order (d major, then side, then b)
    noise_t = nc.dram_tensor("noise_t", [NPAIR, D, 2, BS], BF16, kind="ExternalInput")
    cbf = nc.dram_tensor("cbf", [NUM_EMB, CBW], BF16, kind="ExternalInput")
    oh96 = nc.dram_tensor("oh96", [COLS * NUM_EMB, OHW], BF16, kind="ExternalInput")
    wf32 = nc.dram_tensor("wf32", [D, NPAIR * 4], FP32, kind="ExternalInput")
    wbf = nc.dram_tensor("wbf", [D, NPAIR * 4], BF16, kind="ExternalInput")
    out = nc.dram_tensor("out", [2, BS], FP32, kind="ExternalOutput")

    with tile.TileContext(nc) as tc:
        with (
            tc.tile_pool(name="const", bufs=1) as const_pool,
            tc.tile_pool(name="ms", bufs=1) as ms_pool,
            tc.tile_pool(name="noise", bufs=4) as noise_pool,
            tc.tile_pool(name="tmp", bufs=3) as tmp_pool,
            tc.tile_pool(name="gpsum", bufs=4, space="PSUM") as gath_psum,
            tc.tile_pool(name="opsum", bufs=1, space="PSUM") as out_psum,
            tc.tile_pool(name="osb", bufs=1) as out_sb_pool,
        ):
            # const DMAs split into column ranges -> several parallel queues
            cst = const_pool.tile([NUM_EMB, CBW], BF16, tag="cbf")
            spl = [0, S0, OH0, OH0 + 4 * BS, CBW]
            for si in range(len(spl) - 1):
                nc.sync.dma_start(
                    out=cst[:, spl[si] : spl[si + 1]], in_=cbf[:, spl[si] : spl[si + 1]]
                )
            oh96_sb = const_pool.tile([COLS * NUM_EMB, OHW], BF16, tag="oh96")
            nc.sync.dma_start(out=oh96_sb[:, 0 : OHW // 2], in_=oh96[:, 0 : OHW // 2])
            nc.sync.dma_start(out=oh96_sb[:, OHW // 2 :], in_=oh96[:, OHW // 2 :])
            wf_sb = const_pool.tile([D, NPAIR * 4], FP32, tag="wf32")
            nc.sync.dma_start(out=wf_sb[:], in_=wf32[:])
            wbf_sb = const_pool.tile([D, NPAIR * 4], BF16, tag="wbf")
            nc.sync.dma_start(out=wbf_sb[:], in_=wbf[:])

            mhi_sb = [cst[:, MHI0 + c * D : MHI0 + (c + 1) * D] for c in range(COLS)]
            mlo_sb = [cst[:, MLO0 + c * D : MLO0 + (c + 1) * D] for c in range(COLS)]
            s01_sb = [cst[:, S0 + c * D : S0 + (c + 1) * D] for c in range(COLS)]
            oh_sb = [cst[:, OH0 + c * BS : OH0 + (c + 1) * BS] for c in range(COLS)]
            cmhi_sb = oh96_sb[:, BS : BS + 2]
            cmlo_sb = oh96_sb[:, BS + 2 : BS + 4]
            w_sb = [
                (
                    wf_sb[:, k * 4 : k * 4 + 2],
                    wf_sb[:, k * 4 + 2 : k * 4 + 4],
                )
                for k in range(NPAIR)
            ]
            wbf_parts = [
                (wbf_sb[:, k * 4 : k * 4 + 2], wbf_sb[:, k * 4 + 2 : k * 4 + 4])
                for k in range(NPAIR)
            ]

            # --- HAM warm-up: junk matmuls so the PE clock-gate opens before
            # the real gather/accumulate streams (cold PE runs at 1.2 GHz) ---
            if WARMUP:
                junk = gath_psum.tile([D, CH], FP32, tag="junk", name="junk", bufs=1)
                for wi in range(WARMUP):
                    nc.tensor.matmul(
                        junk[:], s01_sb[0], oh_sb[0][:, 0:CH],
                        start=(wi == 0), stop=(wi == WARMUP - 1),
                    )

            # process pairs so that early pairs only touch early columns; start
            # and end with decomposed pairs (they need no M gathers, so the
            # kernel starts compute earliest and ends on a short chain)
            ksort = sorted(range(NPAIR), key=lambda k: (max(PAIRS[k]), min(PAIRS[k])))
            kdec = [k for k in ksort if pos[k] in (0, 4) and DECOMP]
            kcmb = [k for k in ksort if k not in kdec]
            korder = kdec[:2] + kcmb + kdec[2:]

            # which columns need gathered M (only mul/max/min pairs touch M_g),
            # in order of first use by the sorted pair sequence
            m_cols = []
            for k in korder:
                if pos[k] in (1, 2, 3) or not DECOMP:
                    for c in PAIRS[k]:
                        if c not in m_cols:
                            m_cols.append(c)

            # --- gather S01 (bf16) then M (fp32, hi+lo) per column: [D, BS] ---
            s_g = []
            for c in range(COLS):
                sg = ms_pool.tile([D, BS], BF16, tag=f"sg{c}", name=f"sg{c}")
                for ch in range(NCH):
                    g2 = gath_psum.tile([D, CH], FP32, tag="g", name="g")
                    nc.tensor.matmul(
                        g2[:], s01_sb[c], oh_sb[c][:, bass.ts(ch, CH)],
                        start=True, stop=True,
                    )
                    nc.scalar.copy(sg[:, bass.ts(ch, CH)], g2[:])
                s_g.append(sg)
            m_g = {}
            for c in m_cols:
                mg = ms_pool.tile([D, BS], FP32, tag=f"mg{c}", name=f"mg{c}")
                for ch in range(NCH):
                    g = gath_psum.tile([D, CH], FP32, tag="g", name="g")
                    nc.tensor.matmul(
                        g[:], mhi_sb[c], oh_sb[c][:, bass.ts(ch, CH)],
                        start=True, stop=False,
                    )
                    nc.tensor.matmul(
                        g[:], mlo_sb[c], oh_sb[c][:, bass.ts(ch, CH)],
                        start=False, stop=True,
                    )
                    nc.scalar.copy(mg[:, bass.ts(ch, CH)], g[:])
                m_g[c] = mg

            # --- output accumulators ---
            acc = [
                out_psum.tile([2, CH], FP32, tag=f"acc{ch}", name=f"acc{ch}")
                for ch in range(NCH)
            ]
            any_decomp = any(pos[k] in (0, 4) and DECOMP for k in range(NPAIR))
            n_mm = [0] * NCH  # matmuls expected per chunk, to set stop on last
            for k in range(NPAIR):
                per = 2 if pos[k] in (0, 4) else 1
                for ch in range(NCH):
                    n_mm[ch] += per
            for ch in range(NCH):
                n_mm[ch] += 2 if any_decomp else 0
            done_mm = [0] * NCH

            def acc_mm(ch, lhsT, rhs):
                done_mm[ch] += 1
                nc.tensor.matmul(
                    acc[ch][:], lhsT, rhs,
                    start=(done_mm[ch] == 1),
                    stop=(done_mm[ch] == n_mm[ch]),
                )

            # --- mean path of ALL decomposed pairs: one stacked K=96 matmul
            # per chunk per hi/lo part (columns stacked on the contraction) ---
            if any_decomp:
                for ch in range(NCH):
                    acc_mm(ch, cmhi_sb, oh96_sb[:, bass.ts(ch, CH)])
                    acc_mm(ch, cmlo_sb, oh96_sb[:, bass.ts(ch, CH)])

            # --- pair loop ---
            for k in korder:
                i, j = PAIRS[k]
                l = pos[k]
                # one DMA per noise side: halves first-byte latency and doubles
                # queue parallelism vs a single [D, 2*BS] transfer
                nt = noise_pool.tile([D, 2 * BS], BF16, tag="nt", name="nt")
                nc.sync.dma_start(out=nt[:, 0:BS], in_=noise_t[k, :, 0])
                nc.sync.dma_start(out=nt[:, BS : 2 * BS], in_=noise_t[k, :, 1])
                n0 = nt[:, 0:BS]
                n1 = nt[:, BS : 2 * BS]

                t0 = tmp_pool.tile([D, BS], BF16, tag="t0", name="t0", bufs=4)
                nc.vector.tensor_tensor(t0[:], s_g[i][:], n0, mybir.AluOpType.mult)
                t1 = tmp_pool.tile([D, BS], BF16, tag="t1", name="t1", bufs=4)
                nc.vector.tensor_tensor(t1[:], s_g[j][:], n1, mybir.AluOpType.mult)

                if l in (1, 2, 3) or not DECOMP:
                    p = tmp_pool.tile([D, BS], FP32, tag="p", name="p", bufs=4)
                    nc.vector.tensor_tensor(p[:], t0[:], m_g[i][:], mybir.AluOpType.add)
                    q = tmp_pool.tile([D, BS], FP32, tag="q", name="q", bufs=4)
                    nc.vector.tensor_tensor(q[:], t1[:], m_g[j][:], mybir.AluOpType.add)
                    if l in (1, 2, 3):
                        combo = tmp_pool.tile([D, BS], FP32, tag="combo", name="combo", bufs=5)
                        eng = nc.gpsimd if GPS_COMBO else nc.vector
                        eng.tensor_tensor(combo[:], p[:], q[:], _ALU[l])
                        for ch in range(NCH):
                            acc_mm(ch, w_sb[k][0], combo[:, bass.ts(ch, CH)])
                    else:
                        for ch in range(NCH):
                            acc_mm(ch, w_sb[k][0], p[:, bass.ts(ch, CH)])
                            acc_mm(ch, w_sb[k][1], q[:, bass.ts(ch, CH)])
                else:
                    # noise-path only: out += t0@Wp + t1@Wq
                    # (mean path went through the per-column CM tables above)
                    for ch in range(NCH):
                        acc_mm(ch, wbf_parts[k][0], t0[:, bass.ts(ch, CH)])
                        acc_mm(ch, wbf_parts[k][1], t1[:, bass.ts(ch, CH)])

            # --- write out ---
            osb = out_sb_pool.tile([2, BS], FP32, tag="osb", name="osb")
            for ch in range(NCH):
                nc.scalar.copy(osb[:, bass.ts(ch, CH)], acc[ch][:])
            nc.sync.dma_start(out=out[:], in_=osb[:])

    return nc


def _prepare_inputs(features, emb_mean, emb_std, W_nc, W_cat, log_alpha, noise):
    features = np.asarray(features)
    emb_mean = np.ascontiguousarray(np.asarray(emb_mean, dtype=np.float32))
    emb_std = np.asarray(emb_std, dtype=np.float32)
    W_nc = np.asarray(W_nc, dtype=np.float32)
    W_cat = np.asarray(W_cat, dtype=np.float32)
    log_alpha = np.asarray(log_alpha, dtype=np.float32)
    noise = np.asarray(noise, dtype=np.float32)

    pos = np.argmax(log_alpha, axis=-1).tolist()

    # softplus(emb_std) * 0.01, computed stably on host (tiny tensor)
    s01 = np.logaddexp(0.0, emb_std).astype(np.float32) * np.float32(0.01)

    # one-hot of features: [COLS, NUM_EMB, B]
    onehot = (
        features[:, None, :] == np.arange(NUM_EMB, dtype=features.dtype)[None, :, None]
    ).astype(np.float32)

    # per-pair selected weights as lhsT [D, 2] x 2 parts
    wparts = np.zeros((NPAIR, 2, D, 2), dtype=np.float32)
    for k in range(NPAIR):
        l = pos[k]
        if l == 4:
            wparts[k, 0] = W_cat[k, :, :D].T
            wparts[k, 1] = W_cat[k, :, D:].T
        else:
            wparts[k, 0] = W_nc[k, l].T
            wparts[k, 1] = W_nc[k, l].T

    wf32 = np.zeros((D, NPAIR * 4), dtype=np.float32)
    wbf = np.zeros((D, NPAIR * 4), dtype=BF)
    cm = np.zeros((COLS, NUM_EMB, 2), dtype=np.float32)
    for k in range(NPAIR):
        i, j = PAIRS[k]
        for pi in range(2):
            sl = slice(k * 4 + 2 * pi, k * 4 + 2 * pi + 2)
            wf32[:, sl] = wparts[k, pi]
            wbf[:, sl] = wparts[k, pi].astype(BF)
            if pos[k] in (0, 4) and DECOMP:
                col = i if pi == 0 else j
                cm[col] += emb_mean[col] @ wparts[k, pi]

    # bf16 const pack
    cbf = np.zeros((NUM_EMB, CBW), dtype=BF)
    m_hi = emb_mean.astype(BF)
    m_lo = (emb_mean - m_hi.astype(np.float32)).astype(BF)
    cm_hi = cm.astype(BF)  # [COLS, NUM_EMB, 2]
    cm_lo = (cm - cm_hi.astype(np.float32)).astype(BF)
    for c in range(COLS):
        cbf[:, MHI0 + c * D : MHI0 + (c + 1) * D] = m_hi[c]
        cbf[:, MLO0 + c * D : MLO0 + (c + 1) * D] = m_lo[c]
        cbf[:, S0 + c * D : S0 + (c + 1) * D] = s01[c].astype(BF)

    # oh96 base: stacked CM tables in the last 4 columns (batch-independent)
    oh96_base = np.zeros((COLS * NUM_EMB, OHW), dtype=BF)
    oh96_base[:, BS : BS + 2] = cm_hi.reshape(COLS * NUM_EMB, 2)
    oh96_base[:, BS + 2 : BS + 4] = cm_lo.reshape(COLS * NUM_EMB, 2)

    # noise transposed to [NPAIR, D, 2, B] in bf16
    noise_t = np.ascontiguousarray(noise.transpose(0, 3, 1, 2).astype(BF))

    in_maps = []
    for c in range(NCORES):
        sl = slice(c * BS, (c + 1) * BS)
        cc_arr = cbf.copy()
        oh_arr = oh96_base.copy()
        for col in range(COLS):
            cc_arr[:, OH0 + col * BS : OH0 + (col + 1) * BS] = onehot[col][:, sl]
            oh_arr[col * NUM_EMB : (col + 1) * NUM_EMB, :BS] = onehot[col][:, sl]
        in_maps.append(
            {
                "noise_t": np.ascontiguousarray(noise_t[:, :, :, sl]),
                "cbf": cc_arr,
                "oh96": oh_arr,
                "wf32": wf32,
                "wbf": wbf,
            }
        )
    return pos, in_maps


def _run(inputs: dict, trace: bool = False):
    pos, in_maps = _prepare_inputs(**inputs)
    nc = _build_program(pos)
    nc.finalize()  # Bacc.compile(): wait legalization, reg alloc, etc.
    res = run_bass_kernel_spmd(nc, in_maps, list(range(NCORES)), trace=trace)
    out = np.empty((B, 2), dtype=np.float32)
    for c in range(NCORES):
        out[c * BS : (c + 1) * BS, :] = res.results[c]["out"].T
    return out, res


def kernel(**inputs) -> np.ndarray:
    out, _ = _run(inputs, trace=False)
    return out
